# revision 15
# baseline (speedup 1.0000x reference)
"""Trainium2 Bass kernel for nn_AttentionBlock (GroupNorm + MHA + proj + residual).

Sharding: data-parallel over batch — 8 batch elements, one per NeuronCore.
Each core runs the full block for its batch element; no collectives.

Per-core dataflow (c=512, n=1024, heads=8, d=64, groups=32):
  - x DMA'd as four full-tile [128,1024] transfers (4KB rows), tile 0 first,
    so GroupNorm starts ASAP; weights ride the gpsimd (SWDGE) ring gated
    behind x.
  - GroupNorm per 128-channel tile: bn_stats/bn_aggr (DVE), group aggregation
    + broadcast-back via tiny exact f32 selector matmuls, rstd via DVE
    reciprocal + ScalarE Sqrt; the y = a*x+b pass runs on ScalarE (Identity
    with per-partition scale/bias) to keep DVE short. Sqrt/exp ACT tables are
    each preloaded once by dummy ops pinned off the critical path.
  - qkv as matmuls against host-pre-transposed bf16 weights, emitted in
    [128,512] halves from single-bank PSUM slots. Pair-0 q/k up front;
    everything else (q/k m-tiles, transposed v with fused ones-column) pops
    inside the attention loop during windows where the "B" PSUM tag group is
    provably free (first ~4 mts of each pair, before that pair's second-half
    AV accumulators allocate).
  - Attention software-pipelined over head pairs: per mt the S matmuls and
    exps are emitted first (S feeds ACT with no slack), then the previous
    pair's attention*V accumulation into per-(head, n-half) single-bank
    [65,512] PSUM tiles (tag A = first half, B = second half), then the
    softmax-Z chain once a half completes. Early pairs broadcast 1/Z via a
    DRAM bounce (fully hidden); the last three chains broadcast via a tiny
    PE matmul against a ones-column into free S-slots (no DRAM RTT) so the
    tail never waits on DMA.
  - proj wave A (kt 0..2) interleaved into the pair-3 AV drain; wave B per
    n-half right after that half's normalize; output DMA'd per [128,512]
    tile as it completes.

Host-side algebraic folds (exact):
  - attention scale folded into q weights/bias
  - k bias dropped (row-constant shift is softmax-invariant)
  - v bias folded into proj bias: pb_eff = proj_b + proj_w @ v_b
"""

import sys

for _p in ("/opt/trn_rl_repo", "/root/.axon_site/_ro/trn_rl_repo"):
    if _p not in sys.path:
        sys.path.insert(0, _p)

from contextlib import ExitStack

import ml_dtypes
import numpy as np

import concourse.bass as bass
import concourse.bacc as bacc
import concourse.tile as tile
from concourse import mybir
from concourse.bass_utils import run_bass_kernel_spmd

F32 = mybir.dt.float32
BF16 = mybir.dt.bfloat16
AF = mybir.ActivationFunctionType
OP = mybir.AluOpType

B = 8
C = 512
N = 1024
HEADS = 8
D = 64
GROUPS = 32
GSIZE = C // GROUPS  # 16 channels per group
CT = C // 128  # 4 channel tiles
GPT = GROUPS // CT  # 8 groups per channel tile
NT = N // 128  # 8 spatial tiles
W3 = 3 * C
EPS = 1e-5
NCORES = 8
VW = D + 1  # v columns per head incl. ones column


def _build(nc: bass.Bass):
    x = nc.declare_dram_parameter("x", [C, N], F32, isOutput=False)
    qkvwT = nc.declare_dram_parameter("qkvwT", [C, W3], BF16, isOutput=False)
    projwT = nc.declare_dram_parameter("projwT", [C, C], BF16, isOutput=False)
    biases = nc.declare_dram_parameter("biases", [128, 16], F32, isOutput=False)
    sel = nc.declare_dram_parameter("sel", [128, GPT], F32, isOutput=False)
    selb = nc.declare_dram_parameter("selb", [GPT, 128], F32, isOutput=False)
    out = nc.declare_dram_parameter("out", [C, N], F32, isOutput=True)

    with tile.TileContext(nc) as tc, ExitStack() as ctx:
        singles = ctx.enter_context(tc.tile_pool(name="singles", bufs=1))
        small = ctx.enter_context(tc.tile_pool(name="small", bufs=8))
        work = ctx.enter_context(tc.tile_pool(name="work", bufs=6))
        expp = ctx.enter_context(tc.tile_pool(name="expp", bufs=4))
        drp = ctx.enter_context(tc.tile_pool(name="drp", bufs=4, space="DRAM"))
        # PSUM: psS tag "s" = 2 double-bank S slots; psAB tags "A"/"B" = 2+2
        # single-bank slots (AV half accumulators + deferred qkv transients).
        psS = ctx.enter_context(tc.tile_pool(name="psS", bufs=2, space="PSUM"))
        psAB = ctx.enter_context(tc.tile_pool(name="psAB", bufs=2, space="PSUM"))

        x_sb = singles.tile([128, CT * N], F32)
        y_sb = singles.tile([128, CT * N], BF16)
        q_sb = singles.tile([128, 4 * N], BF16)
        k_sb = singles.tile([128, 4 * N], BF16)
        vplus = singles.tile([128, NT * HEADS * VW], BF16)  # [nt][h][65]
        av_sb = singles.tile([128, CT * N], BF16)
        wqkv_sb = singles.tile([128, CT * W3], BF16)
        wproj_sb = singles.tile([128, CT * C], BF16)
        bias_sb = singles.tile([128, 16], F32)  # 0:4 qb | 4:8 pbeff | 8:12 nw | 12:16 nb
        sel_sb = singles.tile([128, GPT], F32)
        selb_sb = singles.tile([GPT, 128], F32)
        zero_sb = singles.tile([128, 1], F32)
        ones64b = singles.tile([1, 64], BF16)
        ab_sb = singles.tile([128, 2 * CT], F32)  # a cols 0..3, nb2n cols 4..7
        ppart = singles.tile([128, CT * N], F32)
        wsrc = singles.tile([128, 640], BF16)

        nc.vector.memset(zero_sb, 0.0)
        nc.vector.memset(ones64b, 1.0)
        nc.vector.memset(wsrc, 0.0)
        # Only the per-head ones columns need initialising; the v copies
        # overwrite everything else.
        nc.vector.memset(
            vplus[:].rearrange("p (b e) -> p b e", e=VW)[:, :, D:D + 1], 1.0
        )

        # ---- input DMA: x as full tiles (4KB contiguous rows), tile-major so
        # tile 0 lands first; small tables after tile 0; weights on the
        # gpsimd ring gated behind x so they don't steal x bandwidth.
        nc.sync.dma_start(out=bias_sb, in_=biases[:, :])
        nc.sync.dma_start(out=sel_sb, in_=sel[:, :])
        nc.sync.dma_start(out=selb_sb, in_=selb[:, :])
        xdmas = []
        for t in range(CT):
            for hp in range(2):
                rs = t * 128 + hp * 64
                xd = nc.sync.dma_start(
                    out=x_sb[hp * 64:hp * 64 + 64, t * N:(t + 1) * N],
                    in_=x[rs:rs + 64, :],
                )
                if t > 0:
                    tile.add_dep_helper(xd.ins, xdmas[2 * t - 2 + hp].ins,
                                        reason="x tile-serial")
                xdmas.append(xd)
        for t in range(CT):
            cs = slice(t * 128, (t + 1) * 128)
            w1 = nc.gpsimd.dma_start(
                out=wqkv_sb[:, t * W3:(t + 1) * W3], in_=qkvwT[cs, :]
            )
            tile.add_dep_helper(w1.ins, xdmas[3].ins, reason="x t1 before wqkv")
        for t in range(CT):
            cs = slice(t * 128, (t + 1) * 128)
            w2 = nc.gpsimd.dma_start(
                out=wproj_sb[:, t * C:(t + 1) * C], in_=projwT[cs, :]
            )
            tile.add_dep_helper(w2.ins, xdmas[7].ins, reason="x before wproj")

        # sqrt ACT table preload, hidden under the x DMA
        tbl = small.tile([1, 1], F32, tag="tbl", name="tbl")
        nc.scalar.activation(out=tbl, in_=zero_sb[0:1], func=AF.Sqrt,
                             bias=0.0, scale=1.0)

        # PE warm-up through the GroupNorm phase so qkv starts at 2.4 GHz.
        def warmup(n):
            for _ in range(n):
                wp = psAB.tile([128, 512], F32, tag="A", name="warm")
                nc.tensor.matmul(
                    wp, lhsT=wsrc[:, 0:128], rhs=wsrc[:, 128:640],
                    start=True, stop=True,
                )

        warmup(8)

        # ---------------- GroupNorm (per channel tile) ----------------
        y_acts = []
        for t in range(CT):
            warmup(5)
            xt = x_sb[:, t * N:(t + 1) * N]
            st = small.tile([128, 2, 6], F32, tag="bn")
            nc.vector.bn_stats(out=st[:, 0, :], in_=xt[:, 0:512])
            nc.vector.bn_stats(out=st[:, 1, :], in_=xt[:, 512:1024])
            mv = small.tile([128, 2], F32, tag="mv")
            nc.vector.bn_aggr(out=mv, in_=st)
            mv2 = small.tile([128, 2], F32, tag="mv2")  # [mean, mean^2 + var]
            nc.vector.tensor_copy(out=mv2[:, 0:1], in_=mv[:, 0:1])
            nc.vector.tensor_scalar(
                out=mv2[:, 1:2], in0=mv[:, 0:1], scalar1=mv[:, 0:1],
                scalar2=mv[:, 1:2], op0=OP.mult, op1=OP.add,
            )
            # group aggregation via exact f32 selector matmul
            gps = psAB.tile([GPT, 2], F32, tag="B", name=f"gps{t}")
            nc.tensor.matmul(gps, lhsT=sel_sb, rhs=mv2, start=True, stop=True)
            # vvar = E[x^2]_g - Mg^2 + eps ; rstd = sqrt(1/vvar)
            m2g = small.tile([GPT, 1], F32, tag="m2g")
            nc.vector.tensor_scalar(
                out=m2g, in0=gps[:, 0:1], scalar1=gps[:, 0:1], scalar2=EPS,
                op0=OP.mult, op1=OP.subtract,
            )
            vvar = small.tile([GPT, 1], F32, tag="vvar")
            nc.vector.tensor_tensor(out=vvar, in0=gps[:, 1:2], in1=m2g,
                                    op=OP.subtract)
            riv = small.tile([GPT, 1], F32, tag="riv")
            nc.vector.reciprocal(out=riv, in_=vvar)
            gst = small.tile([GPT, 2], F32, tag="gst")  # [M, rstd]
            nc.vector.tensor_copy(out=gst[:, 0:1], in_=gps[:, 0:1])
            nc.scalar.activation(out=gst[:, 1:2], in_=riv, func=AF.Sqrt,
                                 bias=0.0, scale=1.0)
            gbc = psAB.tile([128, 2], F32, tag="B", name=f"gbc{t}")
            nc.tensor.matmul(gbc, lhsT=selb_sb, rhs=gst, start=True, stop=True)
            at = ab_sb[:, t:t + 1]
            nb2n = ab_sb[:, CT + t:CT + t + 1]  # nb - a*M ; y = a*x + nb2n
            nc.vector.tensor_scalar(
                out=at, in0=bias_sb[:, 8 + t:9 + t], scalar1=gbc[:, 1:2],
                scalar2=None, op0=OP.mult,
            )
            amg = small.tile([128, 1], F32, tag="amg")
            nc.vector.tensor_scalar(
                out=amg, in0=at, scalar1=gbc[:, 0:1], scalar2=None, op0=OP.mult,
            )
            nc.vector.tensor_tensor(
                out=nb2n, in0=bias_sb[:, 12 + t:13 + t], in1=amg,
                op=OP.subtract,
            )
            ya = nc.scalar.activation(
                out=y_sb[:, t * N:(t + 1) * N], in_=x_sb[:, t * N:(t + 1) * N],
                func=AF.Identity, bias=nb2n, scale=at,
            )
            y_acts.append(ya)
        # preload the exp table while qkv matmuls run (pinned after the last
        # GroupNorm ACT op so the scheduler can't hoist it)
        te = nc.scalar.activation(out=tbl, in_=zero_sb[0:1], func=AF.Exp,
                                  bias=0.0, scale=1.0)
        tile.add_dep_helper(te.ins, y_acts[0].ins, reason="exp table after y0")

        # ---------------- QKV ----------------
        # PE touchers: absorb the weight-DMA semaphores before the matmuls.
        for kt in range(CT):
            nc.tensor.ldweights(weights=wqkv_sb[0:1, kt * W3:kt * W3 + 1])

        def emit_qkv_half(mt, half, tag):
            # q/k m-tile halves from single-bank PSUM slots
            pp = psAB.tile([128, 512], F32, tag=tag, name=f"pp{mt}_{half}")
            for kt in range(CT):
                nc.tensor.matmul(
                    pp,
                    lhsT=wqkv_sb[:, kt * W3 + mt * 128:kt * W3 + (mt + 1) * 128],
                    rhs=y_sb[:, kt * N + half * 512:kt * N + (half + 1) * 512],
                    start=(kt == 0), stop=(kt == CT - 1),
                )
            if mt < 4:
                dcol = mt * N + half * 512
                nc.vector.tensor_scalar(
                    out=q_sb[:, dcol:dcol + 512], in0=pp,
                    scalar1=bias_sb[:, mt:mt + 1], scalar2=None, op0=OP.add,
                )
            else:
                dcol = (mt - 4) * N + half * 512
                nc.vector.tensor_copy(out=k_sb[:, dcol:dcol + 512], in_=pp)

        def emit_vt(nt, tag):
            # v directly transposed: [n, vrow], ones column pre-set in vplus
            vp = psAB.tile([128, 512], F32, tag=tag, name=f"vp{nt}")
            for kt in range(CT):
                nc.tensor.matmul(
                    vp,
                    lhsT=y_sb[:, kt * N + nt * 128:kt * N + nt * 128 + 128],
                    rhs=wqkv_sb[:, kt * W3 + 2 * C:kt * W3 + 3 * C],
                    start=(kt == 0), stop=(kt == CT - 1),
                )
            dst = vplus[:, nt * HEADS * VW:(nt + 1) * HEADS * VW]
            dst = dst.rearrange("p (h e) -> p h e", e=VW)[:, :, 0:D]
            nc.vector.tensor_copy(out=dst, in_=vp.rearrange("p (h e) -> p h e", e=D))

        # pair-0 q/k up front, kt-major so PE follows the GroupNorm pipeline
        pph = {}
        for mt, half, tag in ((0, 0, "A"), (0, 1, "B"), (4, 0, "A"), (4, 1, "B")):
            pph[(mt, half)] = psAB.tile(
                [128, 512], F32, tag=tag, name=f"pre{mt}_{half}"
            )
        for kt in range(CT):
            for mt in (0, 4):
                for half in range(2):
                    nc.tensor.matmul(
                        pph[(mt, half)],
                        lhsT=wqkv_sb[:, kt * W3 + mt * 128:kt * W3 + (mt + 1) * 128],
                        rhs=y_sb[:, kt * N + half * 512:kt * N + (half + 1) * 512],
                        start=(kt == 0), stop=(kt == CT - 1),
                    )
        for half in range(2):
            nc.vector.tensor_scalar(
                out=q_sb[:, half * 512:half * 512 + 512], in0=pph[(0, half)],
                scalar1=bias_sb[:, 0:1], scalar2=None, op0=OP.add,
            )
            nc.vector.tensor_copy(
                out=k_sb[:, half * 512:half * 512 + 512], in_=pph[(4, half)]
            )

        # deferred work, popped at the end of (pr, mt) bodies in windows where
        # an A/B PSUM slot is provably free: {pr: {mt: [fn]}}
        def _qk(mt, half, tag):
            return lambda: emit_qkv_half(mt, half, tag)

        def _vt(nt, tag):
            return lambda: emit_vt(nt, tag)

        deferred = {
            0: {0: [_vt(0, "A")], 1: [_vt(1, "B"), _vt(2, "A")],
                2: [_vt(3, "B"), _vt(4, "A")], 3: [_vt(5, "B"), _vt(6, "A")],
                4: [_vt(7, "B"), _qk(1, 0, "A")], 5: [_qk(1, 1, "B")],
                6: [_qk(5, 0, "A")], 7: [_qk(5, 1, "B")]},
            1: {0: [_qk(2, 0, "A")], 1: [_qk(2, 1, "A")],
                2: [_qk(6, 0, "A")], 3: [_qk(6, 1, "A")]},
            2: {3: [_qk(3, 0, "A"), _qk(3, 1, "A")],
                6: [_qk(7, 0, "B")], 7: [_qk(7, 1, "B")]},
        }

        def emit_projA(ct, nh2):
            # proj partial over kt 0..2 (pairs 0-2 final well before pair 3)
            ppA = psS.tile([128, 512], F32, tag="s", name=f"pA{ct}_{nh2}")
            for kt in (0, 1, 2):
                nc.tensor.matmul(
                    ppA,
                    lhsT=wproj_sb[:, kt * C + ct * 128:kt * C + (ct + 1) * 128],
                    rhs=av_sb[:, kt * N + nh2 * 512:kt * N + nh2 * 512 + 512],
                    start=(kt == 0), stop=(kt == 2),
                )
            nc.vector.scalar_tensor_tensor(
                out=ppart[:, ct * N + nh2 * 512:ct * N + nh2 * 512 + 512],
                in0=ppA, scalar=bias_sb[:, 4 + ct:5 + ct],
                in1=x_sb[:, ct * N + nh2 * 512:ct * N + nh2 * 512 + 512],
                op0=OP.add, op1=OP.add,
            )

        def emit_projB(ct, nh, tag="s"):
            pool = psS if tag == "s" else psAB
            ppB = pool.tile([128, 512], F32, tag=tag, name=f"pB{ct}_{nh}")
            nc.tensor.matmul(
                ppB,
                lhsT=wproj_sb[:, 3 * C + ct * 128:3 * C + (ct + 1) * 128],
                rhs=av_sb[:, 3 * N + nh * 512:3 * N + nh * 512 + 512],
                start=True, stop=True,
            )
            ob = work.tile([128, 512], F32, tag="ob", name=f"ob{ct}_{nh}")
            nc.vector.tensor_tensor(
                out=ob, in0=ppB,
                in1=ppart[:, ct * N + nh * 512:ct * N + (nh + 1) * 512],
                op=OP.add,
            )
            for hp in range(2):
                nc.sync.dma_start(
                    out=out[ct * 128 + hp * 64:ct * 128 + hp * 64 + 64,
                            nh * 512:(nh + 1) * 512],
                    in_=ob[hp * 64:hp * 64 + 64, :],
                )

        def emit_zchain_dram(p_pr, nh, p_heads, p_apn):
            # wide reciprocal + DRAM-bounce broadcast; ~5.5us RTT, fully
            # hidden behind >=4 mts of pipeline slack for pairs 0-2
            zp = small.tile([64, 16], F32, tag="zp", name=f"zp{p_pr}_{nh}")
            for h in p_heads:
                zrh = small.tile([1, 512], F32, tag="zrh", bufs=4,
                                 name=f"zrh{h}_{nh}")
                nc.vector.tensor_copy(out=zrh, in_=p_apn[(h, nh)][D:D + 1, :])
                nc.sync.dma_start(
                    out=zp[:, (h % 2) * 8:(h % 2) * 8 + 8],
                    in_=zrh.rearrange("o (p j) -> o p j", j=8),
                )
            rzp = small.tile([64, 16], F32, tag="rzp", name=f"rzp{p_pr}_{nh}")
            nc.vector.reciprocal(out=rzp, in_=zp)
            for h in p_heads:
                zd = drp.tile([512], F32, tag="zd", name=f"zd{h}_{nh}")
                nc.sync.dma_start(
                    out=zd, in_=rzp[:, (h % 2) * 8:(h % 2) * 8 + 8]
                )
                rzb = work.tile([D, 512], F32, tag="rzb", name=f"rzb{h}_{nh}")
                nc.sync.dma_start(
                    out=rzb,
                    in_=bass.AP(tensor=zd.tensor, offset=zd.offset,
                                ap=[[0, D], [1, 512]]),
                )
                base = (h % 2) * 64
                nc.vector.tensor_tensor(
                    out=av_sb[base:base + 64,
                              p_pr * N + nh * 512:p_pr * N + nh * 512 + 512],
                    in0=p_apn[(h, nh)][0:D, :], in1=rzb, op=OP.mult,
                )

        def emit_zpe_p1(nh, p_heads, p_apn):
            # pair-3 chain, stage 1: Z row -> 1/Z (approx) -> bf16 row
            state = []
            for h in p_heads:
                zrh = small.tile([1, 512], F32, tag="zrh", bufs=4,
                                 name=f"zrh{h}_{nh}")
                nc.scalar.copy(out=zrh, in_=p_apn[(h, nh)][D:D + 1, :])
                rzr = small.tile([1, 512], F32, tag="rzr", bufs=2,
                                 name=f"rzr{h}_{nh}")
                nc.vector.reciprocal_approx_fast(out=rzr, in_=zrh)
                rzrb = small.tile([1, 512], BF16, tag="rzrb", bufs=2,
                                  name=f"rzrb{h}_{nh}")
                nc.scalar.copy(out=rzrb, in_=rzr)
                state.append((h, rzrb))
            return state

        def emit_zpe_p2(p_pr, nh, p_apn, state):
            # stage 2: PE ones-column broadcast + normalize (no DRAM RTT)
            for h, rzrb in state:
                bc = psS.tile([D, 512], F32, tag="s", name=f"bc{h}_{nh}")
                nc.tensor.matmul(bc, lhsT=ones64b, rhs=rzrb,
                                 start=True, stop=True)
                rzbs = work.tile([D, 512], F32, tag="rzb",
                                 name=f"rzbs{h}_{nh}")
                nc.scalar.copy(out=rzbs, in_=bc)
                base = (h % 2) * 64
                nc.vector.tensor_tensor(
                    out=av_sb[base:base + 64,
                              p_pr * N + nh * 512:p_pr * N + nh * 512 + 512],
                    in0=p_apn[(h, nh)][0:D, :], in1=rzbs, op=OP.mult,
                )

        # ---------------- Attention (software-pipelined over head pairs) ----
        prev = None  # (pr, heads, etiles, apn{(h, nh): tile})
        for pr in range(5):
            if pr < 4:
                heads = (2 * pr, 2 * pr + 1)
                etiles = {h: expp.tile([128, NT * N], BF16, tag="exp",
                                       name=f"exp{h}") for h in heads}
            dq = deferred.get(pr, {})
            zpe = {}
            for mt in range(NT):
                nh = 1 - mt // 4  # second (B) half first: hides both Z RTTs
                sub = mt % 4
                if pr < 4:
                    # S first: exp cadence has no slack. Two heads' qk on
                    # disjoint PE row groups; h-outer so exp(h_even) starts
                    # after its own two matmuls.
                    sps = {h: psS.tile([128, N], F32, tag="s",
                                       name=f"sp{h}_{mt}") for h in heads}
                    for h in heads:
                        base = (h % 2) * 64
                        for nh2 in range(2):
                            nc.tensor.matmul(
                                sps[h][:, nh2 * 512:(nh2 + 1) * 512],
                                lhsT=k_sb[base:base + 64,
                                          pr * N + mt * 128:pr * N + mt * 128 + 128],
                                rhs=q_sb[base:base + 64,
                                         pr * N + nh2 * 512:pr * N + nh2 * 512 + 512],
                                start=True, stop=True,
                                tile_position=(base, 0),
                            )
                        nc.scalar.activation(
                            out=etiles[h][:, mt * N:(mt + 1) * N], in_=sps[h],
                            func=AF.Exp, bias=zero_sb, scale=1.0,
                        )
                if prev is not None:
                    p_pr, p_heads, p_etiles, p_apn = prev
                    for h in p_heads:
                        if sub == 0:
                            p_apn[(h, nh)] = psAB.tile(
                                [VW, 512], F32,
                                tag=("A" if nh == 0 else "B"),
                                name=f"apn{h}_{nh}",
                            )
                        for mq in (2 * sub, 2 * sub + 1):
                            nc.tensor.matmul(
                                p_apn[(h, nh)],
                                lhsT=vplus[:, mq * HEADS * VW + h * VW:
                                           mq * HEADS * VW + (h + 1) * VW],
                                rhs=p_etiles[h][:, mq * N + nh * 512:
                                                mq * N + nh * 512 + 512],
                                start=(sub == 0 and mq == 0),
                                stop=(sub == 3 and mq == 7),
                            )
                    if sub == 3:
                        if p_pr == 3:
                            zpe[nh] = emit_zpe_p1(nh, p_heads, p_apn)
                        else:
                            emit_zchain_dram(p_pr, nh, p_heads, p_apn)
                if pr == 4 and mt >= 4:
                    emit_projA(mt - 4, 1)
                    if mt == 5:
                        emit_zpe_p2(3, 1, prev[3], zpe[1])
                for fn in dq.get(mt, []):
                    fn()
            if pr == 3:
                for kt in range(CT):
                    nc.tensor.ldweights(weights=wproj_sb[0:1, kt * C:kt * C + 1])
            if pr == 4:
                for ct in range(CT):
                    emit_projB(ct, 1, "B")
                emit_zpe_p2(3, 0, prev[3], zpe[0])
                for ct in range(CT):
                    emit_projA(ct, 0)
                for ct in range(CT):
                    emit_projB(ct, 0, "A")
            prev = (pr, heads, etiles, {}) if pr < 4 else None

    return nc


_CACHE = {}


def _get_nc():
    if "nc" not in _CACHE:
        nc = bacc.Bacc()
        _build(nc)
        nc.finalize()
        _CACHE["nc"] = nc
    return _CACHE["nc"]


def prepare_in_maps(x, norm_w, norm_b, qkv_w, qkv_b, proj_w, proj_b):
    x = np.asarray(x, np.float32)
    norm_w = np.asarray(norm_w, np.float32)
    norm_b = np.asarray(norm_b, np.float32)
    qkv_w = np.asarray(qkv_w, np.float32).copy()
    qkv_b = np.asarray(qkv_b, np.float32).copy()
    proj_w = np.asarray(proj_w, np.float32)
    proj_b = np.asarray(proj_b, np.float32)

    scale = D ** -0.5
    qkv_w[:C] *= scale
    qbias = (qkv_b[:C] * scale).astype(np.float32)
    vbias = qkv_b[2 * C:3 * C]
    qkvwT = np.ascontiguousarray(qkv_w.T).astype(ml_dtypes.bfloat16)
    projwT = np.ascontiguousarray(proj_w.T).astype(ml_dtypes.bfloat16)
    pb_eff = (proj_b + proj_w @ vbias).astype(np.float32)

    biases = np.zeros([128, 16], np.float32)
    for t in range(CT):
        biases[:, t] = qbias[t * 128:(t + 1) * 128]
        biases[:, 4 + t] = pb_eff[t * 128:(t + 1) * 128]
        biases[:, 8 + t] = norm_w[t * 128:(t + 1) * 128]
        biases[:, 12 + t] = norm_b[t * 128:(t + 1) * 128]

    sel = np.zeros([128, GPT], np.float32)
    selb = np.zeros([GPT, 128], np.float32)
    for p in range(128):
        g = p // GSIZE  # group index within a channel tile
        sel[p, g] = 1.0 / GSIZE
        selb[g, p] = 1.0
    shared = dict(
        qkvwT=qkvwT, projwT=projwT, biases=biases, sel=sel, selb=selb,
    )
    return [
        dict(x=np.ascontiguousarray(x[i].reshape(C, N)), **shared)
        for i in range(x.shape[0])
    ]


def run(in_maps, trace=False, **kwargs):
    return run_bass_kernel_spmd(
        _get_nc(), in_maps, core_ids=list(range(NCORES)), trace=trace, **kwargs
    )


def kernel(x, norm_w, norm_b, qkv_w, qkv_b, proj_w, proj_b):
    in_maps = prepare_in_maps(x, norm_w, norm_b, qkv_w, qkv_b, proj_w, proj_b)
    res = run(in_maps)
    b, c, h, w = np.asarray(x).shape
    return np.stack(
        [res.results[i]["out"].reshape(c, h, w) for i in range(b)]
    ).astype(np.float32)
